# revision 24
# baseline (speedup 1.0000x reference)
"""AdaIN statistics kernel for TRN2, SPMD across 8 NeuronCores.

Input : f_vol [32, 512, 64, 64] f32
Output: [32, 1024] f32 = concat([mean over (h,w), unbiased std over (h,w)], axis=-1)

Sharding: data-parallel over batch — each of the 8 cores handles 4 batches
([4, 512, 64, 64] shard, 32 MiB). No collectives; the host concatenates the
8 per-core [4, 1024] outputs.

Per core: view the shard as 2048 rows (b*512+c) x 4096 spatial elems.
The shard is streamed in SLABS: a slab with m rows/partition loads
128*m consecutive rows, partition p holding rows base+p*m .. +m (so each
partition's DRAM chunk is m*16 KiB contiguous -> m*16 KiB DMA descriptors;
per-SDMA-engine rate is descriptor-size-bound). m=2 slabs stream at the
HBM cap; m=1 slabs at the end keep the compute tail short.

Raw Bass with manual semaphores (Tile's scheduler emits 2 sync-waits on
slot-reuse DMAs, which this compiler's static-DMA encoding cannot hold):
  SP  : input slab DMAs (ring of 6 x 32 KiB/partition SBUF slots)
  DVE : 8 bn_stats per row + bn_aggr per row -> (mean, biased var)
  ACT : mean copy + sqrt(var * N/(N-1)) per row, output DMAs; ACT also
        computes the FINAL slab itself (Copy/Square+accumulate passes)
        so the last rows don't queue behind DVE's backlog.

DMA completion is not FIFO across in-flight transfers, so each slab gets
its own single-use DMA-completion semaphore. Every cross-instruction data
edge is covered by an explicit semaphore observation so the CoreSim race
detector can verify the design.
"""

from contextlib import ExitStack

import numpy as np

B, C, H, W = 32, 512, 64, 64
N_CORES = 8
B_LOCAL = B // N_CORES  # 4
N = H * W  # 4096
P = 128
ROWS = B_LOCAL * C  # 2048
G = N // 512  # bn_stats groups per row = 8

# rows-per-partition per slab; each slab (128*m rows) must stay inside one
# batch. Consumer: 'dve' = bn_stats path, 'act' = ScalarE accumulate path.
SLABS = [2, 2, 2, 2, 2, 2, 2, 1, 1]
CONSUMER = ["dve"] * 7 + ["act", "dve"]
assert sum(SLABS) * P == ROWS and len(CONSUMER) == len(SLABS)
MMAX = max(SLABS)
NBUF = 6  # input slab ring slots (6 x MMAX*16 KiB/partition)
NSMALL = 2  # stats/mv/res ring slots

_CACHE = {}


def _build():
    import concourse.bass as bass
    from concourse import mybir

    nc = bass.Bass()
    x_ext = nc.declare_dram_parameter(
        "f_vol", [B_LOCAL, C, H, W], mybir.dt.float32, isOutput=False
    )
    out_ext = nc.declare_dram_parameter(
        "out", [B_LOCAL, 2 * C], mybir.dt.float32, isOutput=True
    )

    x = x_ext.ap().rearrange("b c h w -> (b c) (h w)")  # [2048, 4096]

    nslabs = len(SLABS)
    base_rows = [P * sum(SLABS[:j]) for j in range(nslabs)]
    for j, m in enumerate(SLABS):
        assert (base_rows[j] % C) + P * m <= C, f"slab {j} crosses a batch"

    # --- plan: cumulative semaphore targets per slab ---
    # dve_stats: +1 per bn_stats (DVE slabs);  act_stats: +1 per ACT
    # accumulate pass (2 per row, ACT slabs). mv_ready: +1 per bn_aggr.
    # act_done: +2 per DVE-slab row (mean copy + sqrt) or +4 per ACT-slab
    # row (epilogue: mean, square, negate, sqrt) — all on ACT.
    dve_after, act_stats_after, mv_after, actd_after = [], [], [], []
    cd = ca = cm = cact = 0
    for j, m in enumerate(SLABS):
        if CONSUMER[j] == "dve":
            cd += G * m
            cm += m
            cact += 2 * m
        else:
            ca += 2 * m
            cact += 4 * m
        dve_after.append(cd)
        act_stats_after.append(ca)
        mv_after.append(cm)
        actd_after.append(cact)

    with ExitStack() as ctx:
        block = ctx.enter_context(nc.Block(no_gpsimd_drain=True))
        dma_in = [
            ctx.enter_context(nc.semaphore(f"dma_in{j}")) for j in range(nslabs)
        ]
        dma_out = [
            ctx.enter_context(nc.semaphore(f"dma_out{s}")) for s in range(NSMALL)
        ]
        dve_stats = ctx.enter_context(nc.semaphore("dve_stats"))
        act_stats = ctx.enter_context(nc.semaphore("act_stats"))
        mv_ready = ctx.enter_context(nc.semaphore("mv_ready"))
        act_done = ctx.enter_context(nc.semaphore("act_done"))
        xt = ctx.enter_context(
            nc.sbuf_tensor("xt", [P, NBUF, MMAX * N], mybir.dt.float32)
        )
        stats = ctx.enter_context(
            nc.sbuf_tensor("stats", [P, NSMALL, MMAX, G, 6], mybir.dt.float32)
        )
        mv = ctx.enter_context(
            nc.sbuf_tensor("mv", [P, NSMALL, MMAX, 2], mybir.dt.float32)
        )
        res = ctx.enter_context(
            nc.sbuf_tensor("res", [P, NSMALL, 2, MMAX], mybir.dt.float32)
        )
        # ACT-slab accumulators: [sum, sumsq, tmp] per row, no reuse
        acc = ctx.enter_context(
            nc.sbuf_tensor("acc", [P, MMAX, 3], mybir.dt.float32)
        )

        # out-DMA count per slab: m==1 needs two DMAs (per-stat contiguous
        # runs), otherwise one 3D-AP DMA covers both stats
        out_incs = [32 if m == 1 else 16 for m in SLABS]
        out_total = {s: 0 for s in range(NSMALL)}
        out_after = []  # dma_out[j % NSMALL] value after slab j's DMAs land
        for j, m in enumerate(SLABS):
            out_total[j % NSMALL] += out_incs[j]
            out_after.append(out_total[j % NSMALL])

        def slot_free_waits(eng, j):
            """Waits before rewriting xt slot (j % NBUF) for slab j."""
            if j < NBUF:
                return
            jp = j - NBUF
            if CONSUMER[jp] == "dve":
                eng.wait_ge(dve_stats, dve_after[jp])
            else:
                eng.wait_ge(act_stats, act_stats_after[jp])
            eng.wait_ge(dma_in[jp], 16)

        def emit_out_dma(scalar, j, m, s, b, c0):
            if m == 1:
                for q in range(2):
                    dst = bass.AP(
                        tensor=out_ext,
                        offset=b * 2 * C + q * C + c0,
                        ap=[[1, P], [1, 1]],
                    )
                    scalar.dma_start(out=dst, in_=res[:, s, q, 0:1]).then_inc(
                        dma_out[s], 16
                    )
            else:
                dst = bass.AP(
                    tensor=out_ext,
                    offset=b * 2 * C + c0,
                    ap=[[m, P], [C, 2], [1, m]],
                )
                scalar.dma_start(out=dst, in_=res[:, s, :, 0:m]).then_inc(
                    dma_out[s], 16
                )

        @block.sync
        def _(sync):
            for j, m in enumerate(SLABS):
                slot_free_waits(sync, j)
                src = x[base_rows[j] : base_rows[j] + P * m, :].rearrange(
                    "(p m) f -> p (m f)", m=m
                )
                sync.dma_start(out=xt[:, j % NBUF, 0 : m * N], in_=src).then_inc(
                    dma_in[j], 16
                )
            # keep the NEFF alive until every output DMA has landed
            for s in range(NSMALL):
                sync.wait_ge(dma_out[s], out_total[s])

        @block.vector
        def _(vector):
            prev_dve = [jj for jj in range(nslabs) if CONSUMER[jj] == "dve"]
            for j, m in enumerate(SLABS):
                if CONSUMER[j] != "dve":
                    continue
                k = j % NBUF
                s = j % NSMALL
                vector.wait_ge(dma_in[j], 16)
                # stats/mv slot WAR vs the previous DVE slab that used slot s
                pi = prev_dve.index(j)
                jp = None
                for jj in prev_dve[:pi][::-1]:
                    if jj % NSMALL == s:
                        jp = jj
                        break
                if jp is not None:
                    vector.wait_ge(mv_ready, mv_after[jp])
                for r in range(m):
                    for g in range(G):
                        vector.bn_stats(
                            out=stats[:, s, r, g, :],
                            in_=xt[:, k, (r * G + g) * 512 : (r * G + g + 1) * 512],
                        ).then_inc(dve_stats, 1)
                if jp is not None:
                    vector.wait_ge(act_done, actd_after[jp])
                # stats RAW: this slab's bn_stats writes retired
                vector.wait_ge(dve_stats, dve_after[j])
                for r in range(m):
                    vector.bn_aggr(
                        out=mv[:, s, r, :], in_=stats[:, s, r, :, :]
                    ).then_inc(mv_ready, 1)

        @block.scalar
        def _(scalar):
            A = 1.0 / np.sqrt(float(N) * (N - 1))

            def act_accumulate(j, m, k):
                # sum (Copy+accum) then sumsq (Square+accum), in-place on xt
                scalar.wait_ge(dma_in[j], 16)
                base_as = act_stats_after[j] - 2 * m
                for r in range(m):
                    row = xt[:, k, r * N : (r + 1) * N]
                    scalar.activation(
                        out=row,
                        in_=row,
                        func=mybir.ActivationFunctionType.Copy,
                        accum_out=acc[:, r, 0:1],
                    ).then_inc(act_stats, 1)
                    # observe the Copy (xt write + acc[0]) before Square
                    scalar.wait_ge(act_stats, base_as + 2 * r + 1)
                    scalar.activation(
                        out=row,
                        in_=row,
                        func=mybir.ActivationFunctionType.Square,
                        accum_out=acc[:, r, 1:2],
                    ).then_inc(act_stats, 1)

            def act_epilogue(j, m, s, b, c0):
                scalar.wait_ge(act_stats, act_stats_after[j])
                if j >= NSMALL:
                    scalar.wait_ge(dma_out[s], out_after[j - NSMALL])
                ad = actd_after[j] - 4 * m  # running act_done value
                for r in range(m):
                    # mean = sum / N
                    scalar.activation(
                        out=res[:, s, 0, r : r + 1],
                        in_=acc[:, r, 0:1],
                        func=mybir.ActivationFunctionType.Copy,
                        scale=1.0 / N,
                    ).then_inc(act_done, 1)
                    # tmp = (sum*A)^2 = sum^2/(N(N-1))
                    scalar.activation(
                        out=acc[:, r, 2:3],
                        in_=acc[:, r, 0:1],
                        func=mybir.ActivationFunctionType.Square,
                        scale=A,
                    ).then_inc(act_done, 1)
                    ad += 2
                    scalar.wait_ge(act_done, ad)
                    scalar.activation(
                        out=acc[:, r, 2:3],
                        in_=acc[:, r, 2:3],
                        func=mybir.ActivationFunctionType.Copy,
                        scale=-1.0,
                    ).then_inc(act_done, 1)
                    ad += 1
                    scalar.wait_ge(act_done, ad)
                    # std = sqrt(sumsq/(N-1) - sum^2/(N(N-1)))
                    scalar.activation(
                        out=res[:, s, 1, r : r + 1],
                        in_=acc[:, r, 1:2],
                        func=mybir.ActivationFunctionType.Sqrt,
                        scale=1.0 / (N - 1),
                        bias=acc[:, r, 2:3],
                    ).then_inc(act_done, 1)
                    ad += 1
                scalar.wait_ge(act_done, ad)
                emit_out_dma(scalar, j, m, s, b, c0)

            def dve_epilogue(j, m, s, b, c0):
                scalar.wait_ge(mv_ready, mv_after[j])
                if j >= NSMALL:
                    scalar.wait_ge(dma_out[s], out_after[j - NSMALL])
                for r in range(m):
                    scalar.copy(
                        out=res[:, s, 0, r : r + 1], in_=mv[:, s, r, 0:1]
                    ).then_inc(act_done, 1)
                    scalar.activation(
                        out=res[:, s, 1, r : r + 1],
                        in_=mv[:, s, r, 1:2],
                        func=mybir.ActivationFunctionType.Sqrt,
                        scale=float(N) / (N - 1),
                    ).then_inc(act_done, 1)
                # res RAW: this slab's ACT writes retired before DMA reads
                scalar.wait_ge(act_done, actd_after[j])
                emit_out_dma(scalar, j, m, s, b, c0)

            # Emission order: an ACT slab's accumulate is hoisted before the
            # previous slab's epilogue so it starts at DMA arrival instead of
            # queueing behind mv_ready stalls.
            emitted = set()
            for j, m in enumerate(SLABS):
                nj = j + 1
                if (
                    nj < nslabs
                    and CONSUMER[nj] == "act"
                    and nj not in emitted
                ):
                    act_accumulate(nj, SLABS[nj], nj % NBUF)
                    emitted.add(nj)
                s = j % NSMALL
                b, c0 = divmod(base_rows[j], C)
                if CONSUMER[j] == "dve":
                    dve_epilogue(j, m, s, b, c0)
                else:
                    if j not in emitted:
                        act_accumulate(j, m, j % NBUF)
                        emitted.add(j)
                    act_epilogue(j, m, s, b, c0)

    return nc


def kernel(f_vol: np.ndarray) -> np.ndarray:
    from concourse.bass_utils import run_bass_kernel_spmd

    if "nc" not in _CACHE:
        _CACHE["nc"] = _build()
    nc = _CACHE["nc"]

    f_vol = np.ascontiguousarray(f_vol, dtype=np.float32)
    in_maps = [
        {"f_vol": f_vol[i * B_LOCAL : (i + 1) * B_LOCAL]} for i in range(N_CORES)
    ]
    res = run_bass_kernel_spmd(nc, in_maps, core_ids=list(range(N_CORES)))
    return np.concatenate([res.results[i]["out"] for i in range(N_CORES)], axis=0)
